# revision 43
# baseline (speedup 1.0000x reference)
"""Trainium2 Bass kernel for nn_DecoderLayer_56908316672219 (Transformer-XL decoder layer).

v2 sharding (8 cores): core c = (b = c//4, hg = c%4) handles batch b and head
group hg (4 of 16 heads) through attention; at the Wo boundary it switches to
q-sharding: each core computes its Wo row-slice partial x = oT_slice @ Wo_slice
for ALL q rows, a per-q-half ReduceScatter over the quad hands每 core its own
256 q-rows of x (128 from each half), and the core runs the FULL FFN + LN1/LN2
on just those rows (no second collective). Host assembles the 8 row-blocks.

Host staging: all activations/weights pre-transposed + bf16 on the host
(cat/r transposes that burned ~380 PE matmuls in v1 are free now).

Attention: scores stay natural [q, k]; rel_shift via the v1 DRAM G-bounce
(pad bands prefilled once). Head pairs share a 128-partition tile at base
partitions 0/64, so score/G matmuls row-pack (K=64 ×2 concurrent) and PV
matmuls col-pack (M=64 ×2) via tile_position auto-derivation. exp outputs
fp8e4 so the pT transpose matmuls get 4x FWL weight loads.
"""
import contextlib
import math
import os
import numpy as np
import ml_dtypes

import concourse.bass as bass
import concourse.tile as tile
from concourse import bacc, mybir
from concourse.bass_utils import run_bass_kernel_spmd

BF16 = mybir.dt.bfloat16
FP8 = mybir.dt.float8e4
F32 = mybir.dt.float32
AF = mybir.ActivationFunctionType
ALU = mybir.AluOpType

B, QLEN, MLEN, E, H, DH = 2, 1024, 1024, 1024, 16, 64
HD = H * DH
KLEN = QLEN + MLEN          # 2048
LN_EPS = 1e-3
SCALE = 1.0 / math.sqrt(E)  # 1/32
NCORES = 8
HPC = 4                     # heads per core
CSL = HPC * DH              # 256 head-col slice
QH = QLEN // 2              # 512 per q-half

NT = QLEN // 128            # 8 q-row tiles
JT = KLEN // 128            # 16 key tiles
GPAD = -240.0               # fp8e4 max-normal; exp(-240+|AC|) == 0 and no Inf*0 NaN
GW = KLEN + 128             # 2176 padded G row width

_CACHE = {}
LAST_PERF = {}


def _mw(I):
    """valid G width for q-tile I (d-window starts at D0 = 896 - 128*I)."""
    return 1152 + 128 * I


def build_nc():
    nc = bacc.Bacc("TRN2", target_bir_lowering=False, debug=False,
                   num_devices=NCORES)

    def din(name, shape, dtype=BF16):
        return nc.dram_tensor(name, shape, dtype, kind="ExternalInput")

    catT_d = din("catT", [128, 8, KLEN])
    rT_d = din("rT", [128, 8, KLEN])
    wq_d = din("wq_s", [128, 8, CSL])
    wk_d = din("wk_s", [128, 8, CSL])
    wv_d = din("wv_s", [128, 8, CSL])
    wr_d = din("wr_s", [128, 8, CSL])
    wo_d = din("wo_s", [128, 2, E])
    w1_d = din("w1_s", [128, 8, 4 * E])
    w2_d = din("w2_s", [128, 32, E])
    wown_d = din("wown", [128, 2, E])
    bw_d = din("bw_s", [CSL, 1], F32)
    br_d = din("br_s", [CSL, 1], F32)
    g1d = din("g1", [1, E])
    b1d = din("b1", [1, E])
    g2d = din("g2", [1, E])
    b2d = din("b2", [1, E])

    out_d = nc.dram_tensor("out_q", [256, E], F32, kind="ExternalOutput")
    g_dram = nc.dram_tensor("g_scratch", [HPC, NT, 128, GW], FP8)

    id128b_d = nc.inline_tensor(np.eye(128, dtype=ml_dtypes.bfloat16), "id128b_c")
    id128q_d = nc.inline_tensor(np.eye(128, dtype=ml_dtypes.float8_e4m3), "id128q_c")
    ones64_d = nc.inline_tensor(np.ones((1, 64), dtype=ml_dtypes.bfloat16), "ones64_c")
    ones128_d = nc.inline_tensor(np.ones((128, 1), dtype=ml_dtypes.float8_e4m3), "ones128_c")

    rg = [[0, 1, 2, 3], [4, 5, 6, 7]]
    g_blk = 128 * GW
    evac_ctr = [0]

    with tile.TileContext(nc) as tc:
        with tc.tile_pool(name="params", bufs=1) as params, \
             tc.tile_pool(name="psmm", bufs=6, space="PSUM") as psmm, \
             tc.tile_pool(name="ptr", bufs=2, space="PSUM") as ptr, \
             tc.tile_pool(name="dram", bufs=1, space="DRAM") as dram, \
             tc.tile_pool(name="otpool", bufs=1) as otpool, \
             tc.tile_pool(name="xpool", bufs=1) as xpool:

            def evac(dst, src_ps, scale=None):
                """PSUM -> SBUF copy, 2:1 DVE-biased (ACT carries the exps)."""
                if scale is not None:
                    nc.scalar.activation(dst, src_ps, AF.Copy, scale=scale)
                    return
                if evac_ctr[0] % 3 == 2:
                    nc.scalar.activation(dst, src_ps, AF.Copy)
                else:
                    nc.vector.tensor_copy(dst, src_ps)
                evac_ctr[0] += 1

            # ---------------- consts / params (scalar HWDGE queue; the sync
            # queue is reserved for the big catT/weight loads) --------------
            id128b = params.tile([128, 128], BF16, tag="id128b")
            nc.scalar.dma_start(out=id128b[:], in_=id128b_d[:])
            id128q = params.tile([128, 128], FP8, tag="id128q")
            nc.scalar.dma_start(out=id128q[:], in_=id128q_d[:])
            ones64 = params.tile([1, 64], BF16, tag="ones64")
            nc.scalar.dma_start(out=ones64[:], in_=ones64_d[:])
            ones128 = params.tile([128, 1], FP8, tag="ones128")
            nc.scalar.dma_start(out=ones128[:], in_=ones128_d[:])
            epst = params.tile([128, 1], F32, tag="epst")
            nc.vector.memset(epst[:], LN_EPS)

            g1rep = params.tile([128, E], BF16, tag="g1rep")
            b1rep = params.tile([128, E], BF16, tag="b1rep")
            g2rep = params.tile([128, E], BF16, tag="b2rep2")
            b2rep = params.tile([128, E], BF16, tag="b2rep")
            for dst, src in ((g1rep, g1d), (b1rep, b1d), (g2rep, g2d), (b2rep, b2d)):
                nc.scalar.dma_start(
                    out=dst[:], in_=bass.AP(tensor=src, offset=0, ap=[[0, 128], [1, E]]))

            bw_sb = params.tile([128, 2, 1], F32, tag="bw_sb")
            br_sb = params.tile([128, 2, 1], F32, tag="br_sb")
            for tt in range(2):
                nc.scalar.dma_start(out=bw_sb[:, tt, :], in_=bw_d[tt * 128:(tt + 1) * 128, :])
                nc.scalar.dma_start(out=br_sb[:, tt, :], in_=br_d[tt * 128:(tt + 1) * 128, :])

            wown = params.tile([128, 2, E], BF16, tag="wown")
            nc.scalar.dma_start(out=wown[:], in_=wown_d[:])
            wo_sb = params.tile([128, 2, E], BF16, tag="wo_sb")
            nc.scalar.dma_start(out=wo_sb[:], in_=wo_d[:])

            oT_sc = otpool.tile([128, 2, QLEN], BF16, tag="oT_sc")
            xhat_b = xpool.tile([128, 2, E], BF16, tag="xhat_b")
            xT = xpool.tile([128, 8, 256], BF16, tag="xT")

            rs_ins = [dram.tile([QH, E], FP8, name=f"rsi{i}", tag=f"rsi{i}")
                      for i in range(2)]
            rs_outs = [dram.tile([128, E], FP8, name=f"rso{i}", tag=f"rso{i}")
                       for i in range(2)]

            def pe_transpose4(dst_ap3, src_aps, idt):
                """Transpose up to 4 [128,128] tiles through one PSUM bank and
                evacuate with a single strided copy. dst_ap3: [128, n, 128]."""
                pp = ptr.tile([128, 512], F32, tag="ptr")
                for t, s in enumerate(src_aps):
                    nc.tensor.matmul(pp[:, t * 128:(t + 1) * 128], s, idt[:],
                                     start=True, stop=True, skip_group_check=True)
                n = len(src_aps)
                evac(dst_ap3, pp[:, 0:n * 128].rearrange("p (a b) -> p a b", a=n))

            with tc.tile_pool(name="qkv", bufs=1) as qkv:
                qwT = qkv.tile([128, 2, QLEN], BF16, tag="qwT")
                qrT = qkv.tile([128, 2, QLEN], BF16, tag="qrT")
                kT = qkv.tile([128, 2, KLEN], BF16, tag="kT")
                rpT = qkv.tile([128, 2, KLEN], BF16, tag="rpT")
                v_sb = qkv.tile([128, 16, CSL], FP8, tag="v_sb")

                # W1 pool opened before ph12; its load rides the gpsimd queue
                # behind rT and lands during the projections, so it cannot
                # starve the attention phase's latency-critical G-bounce DMAs
                ffn_stack = contextlib.ExitStack()
                ffnw = ffn_stack.enter_context(tc.tile_pool(name="ffnw", bufs=1))
                w1_sb = ffnw.tile([128, 8, 4 * E], BF16, tag="w1_sb")

                # ---------------- phase 1: load + project ----------------
                with tc.tile_pool(name="ph12", bufs=1) as ph12:
                    catT = ph12.tile([128, 8, KLEN], BF16, tag="catT")
                    rT = ph12.tile([128, 8, KLEN], BF16, tag="rT")
                    for et in range(8):
                        nc.sync.dma_start(out=catT[:, et, :], in_=catT_d[:, et, :])
                    for et in range(8):
                        nc.gpsimd.dma_start(out=rT[:, et, :], in_=rT_d[:, et, :])
                    nc.gpsimd.dma_start(out=w1_sb[:], in_=w1_d[:])

                    wq_sb = ph12.tile([128, 8, CSL], BF16, tag="wq_sb")
                    wk_sb = ph12.tile([128, 8, CSL], BF16, tag="wk_sb")
                    wv_sb = ph12.tile([128, 8, CSL], BF16, tag="wv_sb")
                    wr_sb = ph12.tile([128, 8, CSL], BF16, tag="wr_sb")
                    for dst, src in ((wq_sb, wq_d), (wk_sb, wk_d),
                                     (wv_sb, wv_d), (wr_sb, wr_d)):
                        nc.scalar.dma_start(out=dst[:], in_=src[:])

                    # qT (+ biases), two 128-part tiles
                    for tt in range(2):
                        for c in range(2):
                            ps = psmm.tile([128, 512], F32, tag="mm512")
                            for et in range(8):
                                nc.tensor.matmul(
                                    ps[:], wq_sb[:, et, tt * 128:(tt + 1) * 128],
                                    catT[:, et, MLEN + c * 512: MLEN + (c + 1) * 512],
                                    start=(et == 0), stop=(et == 7))
                            sl = slice(c * 512, (c + 1) * 512)
                            nc.vector.tensor_scalar_add(qwT[:, tt, sl], ps[:], bw_sb[:, tt, :])
                            nc.vector.tensor_scalar_add(qrT[:, tt, sl], ps[:], br_sb[:, tt, :])

                    # kT, rpT (1/32-scaled at evacuation)
                    for dst, wsb, rhsT in ((kT, wk_sb, catT), (rpT, wr_sb, rT)):
                        for tt in range(2):
                            for c in range(4):
                                ps = psmm.tile([128, 512], F32, tag="mm512")
                                for et in range(8):
                                    nc.tensor.matmul(
                                        ps[:], wsb[:, et, tt * 128:(tt + 1) * 128],
                                        rhsT[:, et, c * 512:(c + 1) * 512],
                                        start=(et == 0), stop=(et == 7))
                                evac(dst[:, tt, c * 512:(c + 1) * 512], ps[:], scale=SCALE)

                    # v natural [j, 256]
                    for jt in range(16):
                        ps = psmm.tile([128, 512], F32, tag="mm512")
                        for et in range(8):
                            nc.tensor.matmul(ps[:, 0:256], catT[:, et, jt * 128:(jt + 1) * 128],
                                             wv_sb[:, et, :], start=(et == 0), stop=(et == 7))
                        evac(v_sb[:, jt, :], ps[:, 0:256])

                # ------------- phases 2-4 per q-half: attention + Wo + RS ----
                with tc.tile_pool(name="gwr", bufs=2) as gwr, \
                     tc.tile_pool(name="grd", bufs=6) as grd, \
                     tc.tile_pool(name="pnw", bufs=6) as pnw, \
                     tc.tile_pool(name="zw", bufs=2) as zw, \
                     tc.tile_pool(name="ptp", bufs=1) as ptp, \
                     tc.tile_pool(name="xps", bufs=1) as xps:
                    # 4 independent pT streams: (pair h2, head-in-pair hi)
                    pTs = {}
                    for h2 in (0, 2):
                        for hi in range(2):
                            pTs[(h2, hi)] = ptp.tile([128, 16, QH], FP8,
                                                     name=f"pT{h2}{hi}",
                                                     tag=f"pT{h2}{hi}")

                    for half in range(2):
                        I_lo = half * 4
                        lastJ = 11 if half == 0 else 15
                        # one-time zero of fully-masked pT blocks for this half
                        for It in range(4):
                            I = I_lo + It
                            for J in range(I + 9, lastJ + 1):
                                for t in pTs.values():
                                    nc.vector.memset(t[:, J, It * 128:(It + 1) * 128], 0.0)

                        # ---- G build: head pairs row-packed on PE ----
                        for h2 in (0, 2):
                            hp = h2 // 2
                            for It in range(4):
                                I = I_lo + It
                                d0 = 896 - 128 * I
                                mw = _mw(I)
                                slabs = [gwr.tile([128, GW], FP8, name=f"gslab{i}",
                                                  tag=f"gslab{i}") for i in range(2)]
                                for ms in range(0, mw, 512):
                                    cw = min(512, mw - ms)
                                    pss = []
                                    for hi in range(2):
                                        hb = hi * 64
                                        ps = psmm.tile([128, 512], F32, tag="mm512")
                                        nc.tensor.matmul(
                                            ps[:, 0:cw],
                                            qrT[hb:hb + 64, hp, I * 128:(I + 1) * 128],
                                            rpT[hb:hb + 64, hp, d0 + ms:d0 + ms + cw],
                                            start=True, stop=True)
                                        pss.append(ps)
                                    for hi in range(2):
                                        evac(slabs[hi][:, ms:ms + cw], pss[hi][:, 0:cw])
                                for hi, h in enumerate((h2, h2 + 1)):
                                    nc.vector.memset(slabs[hi][:, mw:mw + 128], GPAD)
                                    nc.sync.dma_start(out=g_dram[h, I, :, 0:mw + 128],
                                                      in_=slabs[hi][:, 0:mw + 128])

                        # ---- scores: all 4 heads' chains interleaved so the
                        # strict-FIFO PE queue always has ready work while the
                        # exps run on ACT (keeps HAM at full clock) ----
                        for It in range(4):
                            I = I_lo + It
                            wtot = (I + 9) * 128
                            nch = (wtot + 511) // 512
                            gnats = {}
                            for h2 in (0, 2):
                                for hi in range(2):
                                    g = grd.tile([128, GW], FP8,
                                                 name=f"gnat{h2}{hi}", tag="gnat")
                                    nc.sync.dma_start(
                                        out=g[:, 0:wtot],
                                        in_=bass.AP(tensor=g_dram,
                                                    offset=((h2 + hi) * NT + I) * g_blk + 127,
                                                    ap=[[GW - 1, 128], [1, wtot]]))
                                    gnats[(h2, hi)] = g
                            for jc in range(nch):
                                cw = min(512, wtot - jc * 512)
                                nt_ = cw // 128
                                J0 = (jc * 512) // 128
                                for h2 in (0, 2):
                                    hp = h2 // 2
                                    pss = [psmm.tile([128, 512], F32, name=f"ps{h2}{i}",
                                                     tag="mm512") for i in range(2)]
                                    for hi in range(2):
                                        nc.tensor.matmul(
                                            pss[hi][:, 0:cw], id128q[:],
                                            gnats[(h2, hi)][:, jc * 512:jc * 512 + cw],
                                            start=True, stop=False)
                                    for hi in range(2):
                                        hb = hi * 64
                                        nc.tensor.matmul(
                                            pss[hi][:, 0:cw],
                                            qwT[hb:hb + 64, hp, I * 128:(I + 1) * 128],
                                            kT[hb:hb + 64, hp, jc * 512:jc * 512 + cw],
                                            start=False, stop=True)
                                    for hi in range(2):
                                        pn = pnw.tile([128, 512], FP8, tag="pn")
                                        nc.scalar.activation(pn[:, 0:cw], pss[hi][:, 0:cw],
                                                             AF.Exp)
                                        pe_transpose4(
                                            pTs[(h2, hi)][:, J0:J0 + nt_,
                                                          It * 128:(It + 1) * 128],
                                            [pn[:, t * 128:(t + 1) * 128] for t in range(nt_)],
                                            id128q)

                        for h2 in (0, 2):
                            hp = h2 // 2
                            # ---- Z row-sums via ones-matmul ----
                            zps = [psmm.tile([128, 512], F32, name=f"zps{i}", tag="mm512")
                                   for i in range(2)]
                            for hi in range(2):
                                for J in range(lastJ + 1):
                                    nc.tensor.matmul(
                                        zps[hi][0:1, :], ones128[:], pTs[(h2, hi)][:, J, :],
                                        start=(J == 0), stop=(J == lastJ),
                                        skip_group_check=True)
                            zrow = [zw.tile([1, QH], BF16, name=f"zrow{i}", tag=f"zrow{i}")
                                    for i in range(2)]
                            for hi in range(2):
                                evac(zrow[hi][:], zps[hi][0:1, :])

                            # ---- PV col-packed head pair ----
                            ovps = psmm.tile([128, 512], F32, tag="mm512")
                            for J in range(lastJ + 1):
                                for hi in range(2):
                                    nc.tensor.matmul(
                                        ovps[hi * 64:(hi + 1) * 64, :],
                                        v_sb[:, J, (h2 + hi) * 64:(h2 + hi + 1) * 64],
                                        pTs[(h2, hi)][:, J, :],
                                        start=(J == 0), stop=(J == lastJ),
                                        skip_group_check=True)
                            zrep = psmm.tile([128, 512], F32, tag="mm512")
                            nc.tensor.matmul(zrep[0:64, :], ones64[:], zrow[0][:, :],
                                             start=True, stop=True, skip_group_check=True)
                            nc.tensor.matmul(zrep[64:128, :], ones64[:], zrow[1][:, :],
                                             start=True, stop=True, skip_group_check=True)
                            zrs = zw.tile([128, 512], F32, tag="zrs")
                            nc.vector.reciprocal(zrs[:], zrep[:])
                            qsl = slice(half * QH, (half + 1) * QH)
                            for hi in range(2):
                                hb = hi * 64
                                nc.vector.tensor_tensor(
                                    oT_sc[hb:hb + 64, hp, qsl],
                                    ovps[hb:hb + 64, :], zrs[hb:hb + 64, :], ALU.mult)

                        # ---- Wo partial: x_part[qhalf, E] -> fp8 RS over quad
                        # (partials are ~1e-2 scale; x64 keeps fp8e4 in range)
                        xp_sb = xps.tile([128, 4, E], FP8, tag="xp_sb")
                        for It in range(4):
                            I = I_lo + It
                            for c in range(2):
                                ps = psmm.tile([128, 512], F32, tag="mm512")
                                for hp in range(2):
                                    nc.tensor.matmul(
                                        ps[:], oT_sc[:, hp, I * 128:(I + 1) * 128],
                                        wo_sb[:, hp, c * 512:(c + 1) * 512],
                                        start=(hp == 0), stop=(hp == 1))
                                nc.scalar.activation(xp_sb[:, It, c * 512:(c + 1) * 512],
                                                     ps[:], AF.Copy, scale=64.0)
                            nc.sync.dma_start(
                                out=rs_ins[half][It * 128:(It + 1) * 128, :],
                                in_=xp_sb[:, It, :])
                        nc.gpsimd.collective_compute(
                            "ReduceScatter", ALU.add, ins=[rs_ins[half].opt()],
                            outs=[rs_outs[half].opt()], replica_groups=rg)

                # ---------------- phase 5+6: LN1, xhatT, W1 per half --------
                # (half 0's LN1 + W1 pass overlaps half 1's in-flight RS)
                with tc.tile_pool(name="lnw", bufs=2) as lnw, \
                     tc.tile_pool(name="ffn2", bufs=1) as ffn2, \
                     tc.tile_pool(name="big56", bufs=2) as big56:
                    h1T = ffn2.tile([128, 32, 256], BF16, tag="h1T")
                    for half in range(2):
                        arr = big56.tile([128, E], FP8, tag="arr")
                        nc.sync.dma_start(out=arr[:], in_=rs_outs[half][:])
                        x = big56.tile([128, E], F32, tag="xrow")
                        nc.vector.scalar_tensor_tensor(
                            x[:], arr[:], 1.0 / 64.0, wown[:, half, :],
                            ALU.mult, ALU.add)
                        _layernorm(nc, lnw, x, g1rep, b1rep, None,
                                   xhat_b[:, half, :], epst)
                        for eg in range(2):
                            pe_transpose4(
                                xT[:, eg * 4:(eg + 1) * 4, half * 128:(half + 1) * 128],
                                [xhat_b[:, half, (eg * 4 + t) * 128:(eg * 4 + t + 1) * 128]
                                 for t in range(4)], id128b)
                        qs = slice(half * 128, (half + 1) * 128)
                        for mc in range(32):
                            ps = psmm.tile([128, 512], F32, tag="mm512")
                            for et in range(8):
                                nc.tensor.matmul(
                                    ps[:, 0:128], w1_sb[:, et, mc * 128:(mc + 1) * 128],
                                    xT[:, et, qs], start=(et == 0), stop=(et == 7))
                            if mc % 2 == 0:
                                nc.scalar.activation(h1T[:, mc, qs], ps[:, 0:128], AF.Relu)
                            else:
                                nc.vector.tensor_scalar_max(h1T[:, mc, qs], ps[:, 0:128], 0.0)

                    # W2 streamed in 8 chunks; y accumulates in 4 psum banks
                    yps = [psmm.tile([128, 512], F32, name=f"yps{i}", tag="mm512")
                           for i in range(4)]
                    with tc.tile_pool(name="w2s", bufs=2) as w2s:
                        for ch in range(8):
                            w2c = w2s.tile([128, 4, E], BF16, tag="w2c")
                            nc.gpsimd.dma_start(out=w2c[:], in_=w2_d[:, ch * 4:(ch + 1) * 4, :])
                            for ct in range(4):
                                c = ch * 4 + ct
                                for qt in range(2):
                                    for ec in range(2):
                                        nc.tensor.matmul(
                                            yps[qt * 2 + ec][:],
                                            h1T[:, c, qt * 128:(qt + 1) * 128],
                                            w2c[:, ct, ec * 512:(ec + 1) * 512],
                                            start=(c == 0), stop=(c == 31),
                                            skip_group_check=True)

                    for qt in range(2):
                        z = big56.tile([128, E], F32, tag="zrow")
                        for ec in range(2):
                            nc.vector.tensor_tensor(
                                z[:, ec * 512:(ec + 1) * 512], yps[qt * 2 + ec][:],
                                xhat_b[:, qt, ec * 512:(ec + 1) * 512], ALU.add)
                        o = big56.tile([128, E], F32, tag="orow")
                        _layernorm(nc, lnw, z, g2rep, b2rep, o, None, epst)
                        nc.sync.dma_start(out=out_d[qt * 128:(qt + 1) * 128, :], in_=o[:])
                ffn_stack.close()

    nc.compile()
    return nc


def _layernorm(nc, pool, x, grep, brep, out_f32, out_b16, epst):
    """LayerNorm along free axis (E) of one [128, E] f32 tile."""
    mu = pool.tile([128, 1], F32, tag="ln_mu")
    nc.vector.tensor_reduce(mu[:], x[:], mybir.AxisListType.X, ALU.add)
    mun = pool.tile([128, 1], F32, tag="ln_mun")
    nc.scalar.activation(mun[:], mu[:], AF.Copy, scale=1.0 / E)
    xc = pool.tile([128, E], F32, tag="ln_xc")
    nc.vector.tensor_scalar_sub(xc[:], x[:], mun[:])
    sq = pool.tile([128, E], F32, tag="ln_sq")
    vs = pool.tile([128, 1], F32, tag="ln_vs")
    nc.scalar.activation(sq[:], xc[:], AF.Square, accum_out=vs[:])
    sd = pool.tile([128, 1], F32, tag="ln_sd")
    nc.scalar.activation(sd[:], vs[:], AF.Sqrt, scale=1.0 / E, bias=epst[:])
    rstd = pool.tile([128, 1], F32, tag="ln_rstd")
    nc.vector.reciprocal(rstd[:], sd[:])
    tmp = pool.tile([128, E], F32, tag="ln_tmp")
    nc.vector.scalar_tensor_tensor(tmp[:], xc[:], rstd[:], grep[:], ALU.mult, ALU.mult)
    if out_f32 is not None:
        nc.vector.tensor_tensor(out_f32, tmp[:], brep[:], ALU.add)
        if out_b16 is not None:
            nc.vector.tensor_copy(out_b16, out_f32)
    else:
        nc.vector.tensor_tensor(out_b16, tmp[:], brep[:], ALU.add)


# ---------------------------------------------------------------------------
# host driver
# ---------------------------------------------------------------------------

def _np_reference(w, r, member, attn_mask, Wq, Wk, Wv, Wr, Wo, r_w_bias, r_r_bias,
                  ln1_g, ln1_b, W1, W2, ln2_g, ln2_b):
    """Exact numpy fallback (used only if attn_mask is not the causal mask)."""
    def ln(x, g, b):
        mu = x.mean(-1, keepdims=True)
        var = ((x - mu) ** 2).mean(-1, keepdims=True)
        return (x - mu) / np.sqrt(var + LN_EPS) * g + b

    b_, qlen, e = w.shape
    h, dh = r_w_bias.shape
    cat = np.concatenate([member, w], axis=1)
    q = (cat @ Wq)[:, -qlen:]
    k = cat @ Wk
    v = cat @ Wv
    rp = (r @ Wr)[0]
    qh = q.reshape(b_, qlen, h, dh)
    kh = k.reshape(b_, -1, h, dh)
    vh = v.reshape(b_, -1, h, dh)
    rh = rp.reshape(-1, h, dh)
    AC = np.einsum('bqhd,bkhd->bhqk', qh + r_w_bias, kh)
    BD = np.einsum('bqhd,khd->bhqk', qh + r_r_bias, rh)
    bb, hh, qq, kk = BD.shape
    BD = np.pad(BD, ((0, 0), (0, 0), (0, 0), (1, 0)))
    BD = BD.reshape(bb, hh, kk + 1, qq)[:, :, 1:, :].reshape(bb, hh, qq, kk)
    attn = (AC + BD) / np.sqrt(np.float32(e))
    m = attn_mask[None, None]
    attn = attn * (1.0 - m) - 1e30 * m
    attn = attn - attn.max(-1, keepdims=True)
    ex = np.exp(attn)
    p = ex / ex.sum(-1, keepdims=True)
    o = np.einsum('bhqk,bkhd->bqhd', p, vh).reshape(b_, qlen, h * dh)
    o = o @ Wo
    x = ln(w + o, ln1_g, ln1_b)
    y = np.maximum(x @ W1, 0.0) @ W2
    return ln(y + x, ln2_g, ln2_b).astype(np.float32)


def _t8(a, n):
    """[n*128, X] -> [128, n, X] (partition-major tiling), bf16."""
    x = np.ascontiguousarray(a, dtype=np.float32).reshape(n, 128, a.shape[-1])
    return np.ascontiguousarray(
        x.transpose(1, 0, 2)).astype(ml_dtypes.bfloat16)


def make_in_maps(inp):
    c = np.ascontiguousarray
    bf = ml_dtypes.bfloat16
    in_maps = []
    r0 = np.asarray(inp["r"][0], np.float32)
    rT = _t8(r0.T, 8)                                 # [128, 8, KLEN]
    w1t = _t8(np.asarray(inp["W1"], np.float32), 8)   # [128, 8, 4096]
    w2t = _t8(np.asarray(inp["W2"], np.float32), 32)  # [128, 32, 1024]
    catTs, wbs = {}, {}
    for b in range(B):
        cat = np.concatenate([inp["member"][b], inp["w"][b]], axis=0)
        catTs[b] = _t8(np.asarray(cat, np.float32).T, 8)
        wbs[b] = np.asarray(inp["w"][b], np.float32)
    for core in range(NCORES):
        b, hg = core // 4, core % 4
        cs = slice(hg * CSL, (hg + 1) * CSL)
        wown = np.stack([wbs[b][hg * 128:hg * 128 + 128],
                         wbs[b][512 + hg * 128: 512 + hg * 128 + 128]],
                        axis=1).astype(bf)            # [128, 2, E]
        in_maps.append({
            "catT": catTs[b],
            "rT": rT,
            "wq_s": _t8(inp["Wq"][:, cs], 8),
            "wk_s": _t8(inp["Wk"][:, cs], 8),
            "wv_s": _t8(inp["Wv"][:, cs], 8),
            "wr_s": _t8(inp["Wr"][:, cs], 8),
            "wo_s": _t8(inp["Wo"][cs, :], 2),
            "w1_s": w1t,
            "w2_s": w2t,
            "wown": c(wown),
            "bw_s": c(inp["r_w_bias"][hg * HPC:(hg + 1) * HPC].reshape(CSL, 1)),
            "br_s": c(inp["r_r_bias"][hg * HPC:(hg + 1) * HPC].reshape(CSL, 1)),
            "g1": c(inp["ln1_g"].reshape(1, E)).astype(bf),
            "b1": c(inp["ln1_b"].reshape(1, E)).astype(bf),
            "g2": c(inp["ln2_g"].reshape(1, E)).astype(bf),
            "b2": c(inp["ln2_b"].reshape(1, E)).astype(bf),
        })
    return in_maps


def kernel(**inputs):
    inp = {k: np.asarray(v, dtype=np.float32) for k, v in inputs.items()}
    causal = (np.arange(KLEN)[None, :] > (np.arange(QLEN)[:, None] + MLEN)).astype(np.float32)
    if not np.array_equal(inp["attn_mask"], causal):
        return _np_reference(**inp)

    if "nc" not in _CACHE:
        _CACHE["nc"] = build_nc()
    nc = _CACHE["nc"]

    in_maps = make_in_maps(inp)
    trace = bool(int(os.environ.get("BASS_KERNEL_TRACE", "0")))
    res = run_bass_kernel_spmd(nc, in_maps, core_ids=list(range(NCORES)), trace=trace)
    LAST_PERF["exec_time_ns"] = res.exec_time_ns
    LAST_PERF["trace"] = res.instructions_and_trace
    out = np.empty((B, QLEN, E), np.float32)
    for core in range(NCORES):
        b, hg = core // 4, core % 4
        oq = res.results[core]["out_q"]
        for half in range(2):
            r0 = half * 512 + hg * 128
            out[b, r0:r0 + 128] = oq[half * 128:(half + 1) * 128]
    return out
